# revision 6
# baseline (speedup 1.0000x reference)
"""DeepSeek-style sparse attention (causal + local-window softmax mix) on 8 trn2 cores.

v2: bf16 datapath + host-side x transpose + software-pipelined emission.

Sharding: tensor-parallel over heads. 16 Q heads / 4 KV heads; core c gets
Q heads {2c, 2c+1} and their shared KV head c//2. Each core computes a
partial output projection (its 256 rows of wo); the host sums the 8 partials.

Differences from v1:
- x is transposed and cast to bf16 on the HOST: no PE transposes, no
  PSUM->SBUF staging copies, half the x DMA traffic.
- All matmuls run in bf16 (same PE rate as fp32r at >=256-wide rhs, but
  1.0 cycles/row at any width, half the SBUF/DMA footprint).
- v is produced directly in [t, d] layout by using x^T chunks as the
  stationary operand (no v transposes).
- exp() runs on [128, 512/1024] PSUM spans (kb pairs/quads) to amortize
  the ~400-cycle Activation-engine access bubble.
- Attention emission is software-pipelined: score matmuls for chunk j+1
  are emitted before the PV/sum matmuls of chunk j, so the in-order PE
  never stalls on the Act-engine exp.
- Causal and local softmax share exp(S) tiles; sums via ones-matmuls into
  one [1,512] PSUM tile (causal|local), one reciprocal + one partition
  broadcast + a 512-wide multiply per group.
- Output partials are written bf16 (host sums in float64).
"""

import math

import numpy as np
import ml_dtypes

import concourse.bass as bass
import concourse.mybir as mybir
import concourse.tile as tile
from concourse import bacc
from concourse.bass_utils import run_bass_kernel_spmd

P = 128
T = 2048
C = 2048
D = 128                 # head dim
N_HEAD = 16
N_KV_HEAD = 4
H_LOC = 2               # q heads per core
N_CORES = 8
TB = T // P             # 16 t blocks of 128
KC = C // P             # 16 contraction chunks of 128
QT = T // 512           # 4 t quarters (projection phase)
G = T // 256            # 8 query groups of 256 (attention phase)
NEG = -1.0e30
F32 = mybir.dt.float32
BF16 = mybir.dt.bfloat16
BF = ml_dtypes.bfloat16

_PROGRAM_CACHE = {}


def _host_constants():
    """Host-precomputed constant tensors shipped as kernel inputs."""
    i = np.arange(P)
    tril = (i[:, None] <= i[None, :])          # [k_loc, q_loc]: valid iff k <= q
    mc_add = np.where(tril, 0.0, NEG).astype(np.float32)
    mc_mul = tril.astype(np.float32)
    ma_add = np.concatenate([mc_add, np.zeros((P, P), np.float32)], axis=1)
    mb_add = np.concatenate([np.full((P, P), NEG, np.float32), mc_add], axis=1)
    ma_mul = np.concatenate([mc_mul, np.ones((P, P), np.float32)], axis=1)
    mb_mul = np.concatenate([np.zeros((P, P), np.float32), mc_mul], axis=1)

    inv_freq = 1.0 / (10000.0 ** (np.arange(0, D, 2, dtype=np.float64) / D))
    t = np.arange(T, dtype=np.float64)
    freqs = t[:, None] * inv_freq[None, :]              # [T, D/2]
    emb = np.concatenate([freqs, freqs], axis=-1)       # [T, D]
    cos_t = np.cos(emb).T.astype(np.float32).copy()     # [D, T]
    sin_t = np.sin(emb).T.astype(np.float32).copy()

    # rot matmul weights: lhsT[d, d'] with out[d'] = -q[d'+64] (d'<64), q[d'-64] (d'>=64)
    rot_t = np.zeros((P, P), np.float32)
    rot_t[64 + np.arange(64), np.arange(64)] = -1.0
    rot_t[np.arange(64), 64 + np.arange(64)] = 1.0

    return {
        "mask_add": np.concatenate([ma_add, mc_add], axis=1),     # [P, 384] f32
        "mul_cat": np.concatenate([ma_mul, mb_mul], axis=1).astype(BF),
        "cos_b": cos_t.astype(BF), "sin_t": sin_t,
        "rot_t": rot_t.astype(BF),
        "ones_c": np.ones((P, 1), BF),
    }


def _emit(nc):
    # xf/wqf/wkvf are host-relaid-out so every DMA is long contiguous runs:
    # xf rows are SBUF partitions, cols are (kc, t) for one t-quarter
    xf = nc.dram_tensor("xf", [QT * P, KC * 512], BF16,
                        kind="ExternalInput").ap()
    wqf = nc.dram_tensor("wqf", [P, KC * H_LOC * D], BF16,
                         kind="ExternalInput").ap()
    wkvf = nc.dram_tensor("wkvf", [P, KC * 2 * D], BF16,
                          kind="ExternalInput").ap()
    wo = nc.dram_tensor("wo", [H_LOC * D, C], BF16, kind="ExternalInput").ap()
    cos_b_d = nc.dram_tensor("cos_b", [P, T], BF16, kind="ExternalInput").ap()
    sin_d = nc.dram_tensor("sin_t", [P, T], F32, kind="ExternalInput").ap()
    rot_d = nc.dram_tensor("rot_t", [P, P], BF16, kind="ExternalInput").ap()
    ones_d = nc.dram_tensor("ones_c", [P, 1], BF16, kind="ExternalInput").ap()
    mask_d = nc.dram_tensor("mask_add", [P, 384], F32, kind="ExternalInput").ap()
    mul_d = nc.dram_tensor("mul_cat", [P, 512], BF16, kind="ExternalInput").ap()
    out = nc.dram_tensor("out", [T, C], BF16, kind="ExternalOutput").ap()

    Exp = mybir.ActivationFunctionType.Exp

    with tile.TileContext(nc) as tc:
        from contextlib import ExitStack
        with ExitStack() as ctx:
            # DMA queue plan: projection weights FIRST on the scalar queue
            # (they gate the first PE matmul), small consts after; cos/sin
            # (needed ~10us in) then wo (needed ~halfway) on the vector
            # queue; x chunks + out stores on the sync queue.
            consts = ctx.enter_context(tc.tile_pool(name="consts", bufs=1))
            # one TILE per 4-kc chunk: tile-granular dependency tracking
            # means a single big tile would gate the first matmul on the
            # last chunk's DMA
            wq_t = []
            wkv_t = []
            kc2t = {}
            bounds = [(0, 1), (1, 5), (5, 9), (9, 13), (13, 16)]
            for ti, (lo_kc, hi_kc) in enumerate(bounds):
                n = hi_kc - lo_kc
                wqc = consts.tile([P, n, H_LOC * D], BF16, name=f"wq{ti}")
                nc.scalar.dma_start(
                    out=wqc, in_=wqf[:, lo_kc * 256:hi_kc * 256])
                wq_t.append(wqc)
                wkvc = consts.tile([P, n, 2 * D], BF16, name=f"wkv{ti}")
                nc.scalar.dma_start(
                    out=wkvc, in_=wkvf[:, lo_kc * 256:hi_kc * 256])
                wkv_t.append(wkvc)
                for kc in range(lo_kc, hi_kc):
                    kc2t[kc] = (ti, kc - lo_kc)

            def wq_sl(kc, lo, hi):
                ti, off = kc2t[kc]
                return wq_t[ti][:, off, lo:hi]

            def wkv_sl(kc, lo, hi):
                ti, off = kc2t[kc]
                return wkv_t[ti][:, off, lo:hi]
            rot_sb = consts.tile([P, P], BF16)
            nc.scalar.dma_start(out=rot_sb, in_=rot_d)
            ones = consts.tile([P, 1], BF16)
            nc.scalar.dma_start(out=ones, in_=ones_d)
            maskc = consts.tile([P, 384], F32)
            nc.scalar.dma_start(out=maskc, in_=mask_d)
            mulc = consts.tile([P, 512], BF16)
            nc.scalar.dma_start(out=mulc, in_=mul_d)
            cos_sb = consts.tile([P, T], BF16)
            nc.scalar.dma_start(out=cos_sb, in_=cos_b_d)
            sin_sb = consts.tile([P, T], F32)
            nc.scalar.dma_start(out=sin_sb, in_=sin_d)

            # persistent activations, split per quarter/group so readers
            # depend only on the slice they touch (tile-granular deps)
            persist = ctx.enter_context(tc.tile_pool(name="persist", bufs=1))
            qT_t = [[persist.tile([P, 512], BF16, name=f"qT{h}_{q}")
                     for q in range(QT)] for h in range(H_LOC)]
            kT_t = [persist.tile([P, 512], BF16, name=f"kT_{q}")
                    for q in range(QT)]
            v_t = [persist.tile([P, 512], BF16, name=f"v_{q}")
                   for q in range(QT)]
            wo_sb = persist.tile([P, H_LOC, C], BF16)
            nc.scalar.dma_start(
                out=wo_sb, in_=wo.rearrange("(h p) n -> p h n", p=P))
            outT_t = [[persist.tile([P, 256], BF16, name=f"oT{h}_{g}")
                       for g in range(G)] for h in range(H_LOC)]

            def kT_sl(kb):
                return kT_t[kb // 4][:, (kb % 4) * P:(kb % 4 + 1) * P]

            def v_sl(kb):
                return v_t[kb // 4][:, (kb % 4) * D:(kb % 4 + 1) * D]

            def qT_sl(h, g):
                return qT_t[h][g // 2][:, (g % 2) * 256:(g % 2 + 1) * 256]

            def outT_sl(h, tb):
                return outT_t[h][tb // 2][:, (tb % 2) * P:(tb % 2 + 1) * P]

            # ---------------- phase P: projections + rope -------------------
            with ExitStack() as pctx:
                # PE pstate warm-up: the clock needs ~3us of continuous
                # execution to reach 2.4GHz, and the PE would otherwise sit
                # idle through the ~4.5us DMA prologue and then pay the ramp
                # on real work. Run dummy matmuls on an iota-seeded scratch
                # tile (Pool engine is idle at t=0; Act is busy dispatching
                # DMAs), and keep sprinkling dummies between the DMA-paced
                # early kc-groups so waits never reset the ramp.
                warm = pctx.enter_context(tc.tile_pool(name="warm", bufs=1))
                wsb = warm.tile([P, P], BF16)
                nc.gpsimd.iota(wsb, [[1, P]], base=0, channel_multiplier=0,
                               allow_small_or_imprecise_dtypes=True)


                xpool = pctx.enter_context(tc.tile_pool(name="xpool", bufs=3))
                ps_proj = pctx.enter_context(
                    tc.tile_pool(name="ps_proj", bufs=1, space="PSUM"))
                # bufs=1 is safe: ropes of one quarter are separated by a
                # full projection chain, so prot's reader is long done before
                # the next rot matmul reuses the bank
                ps_rot = pctx.enter_context(
                    tc.tile_pool(name="ps_rot", bufs=1, space="PSUM"))
                raws = pctx.enter_context(tc.tile_pool(name="raws", bufs=4))
                t12 = pctx.enter_context(tc.tile_pool(name="t12", bufs=3))

                def rope(ps_raw, dst, tsl):
                    raw = raws.tile([P, 512], BF16, tag="raw", name="raw")
                    nc.scalar.copy(raw, ps_raw)
                    prot = ps_rot.tile([P, 512], F32, tag="prot", name="prot")
                    nc.tensor.matmul(prot, rot_sb, raw, start=True, stop=True)
                    t1 = t12.tile([P, 512], F32, tag="t1", name="t1")
                    nc.vector.tensor_mul(t1, prot, sin_sb[:, tsl])
                    t2 = t12.tile([P, 512], F32, tag="t2", name="t2")
                    nc.vector.tensor_mul(t2, raw, cos_sb[:, tsl])
                    nc.vector.tensor_add(dst, t1, t2)

                def load_xq(qq):
                    xq = []
                    rows = slice(qq * P, (qq + 1) * P)
                    for j4 in range(4):
                        xc = xpool.tile([P, 4, 512], BF16, tag=f"xq{j4}",
                                        name=f"xq{j4}")
                        nc.sync.dma_start(
                            out=xc, in_=xf[rows, j4 * 2048:(j4 + 1) * 2048])
                        xq.append(xc)
                    return xq

                def xq_sl(xq, kc):
                    return xq[kc // 4][:, kc % 4, :]

                def xq_slj(xq, kc, jsl):
                    return xq[kc // 4][:, kc % 4, jsl]

                pwarm = ps_proj.tile([P, 512], F32, tag="pq0", name="pq0")

                def warm_pe(n):
                    # scratch matmuls into the pq0 bank; the real chain's
                    # start=True lazy-zero erases them
                    for _ in range(n):
                        nc.tensor.matmul(pwarm[:, 0:P], wsb, wsb,
                                         start=True, stop=True)

                warm_pe(50)

                def proj_tiles():
                    pq0 = ps_proj.tile([P, 512], F32, tag="pq0", name="pq0")
                    pq1 = ps_proj.tile([P, 512], F32, tag="pq1", name="pq1")
                    pk = ps_proj.tile([P, 512], F32, tag="pk", name="pk")
                    pv = ps_proj.tile([P, 512], F32, tag="pv", name="pv")
                    return pq0, pq1, pk, pv

                def emit_prev_rope(prev, i):
                    # rope/v-copy of the previous quarter's tensor i, emitted
                    # just before this quarter's chain reuses its PSUM bank
                    if prev is None:
                        return
                    pq0, pq1, pk, pv, ptsl, pqq = prev
                    if i == 0:   # kT first: phase A's first scores need it
                        rope(pk, kT_t[pqq], ptsl)
                    elif i == 1:
                        rope(pq0, qT_t[0][pqq], ptsl)
                    elif i == 2:
                        nc.scalar.copy(v_t[pqq], pv)
                    else:
                        rope(pq1, qT_t[1][pqq], ptsl)

                # quarter 0: kc-interleaved so PE consumption matches x DMA
                # arrival order (no pstate-resetting stalls at warmup)
                xq = load_xq(0)
                pq0, pq1, pk, pv = proj_tiles()
                for kc in range(KC):
                    if kc == 4:
                        warm_pe(4)   # bridge the wkv0-wait gap; keep clock up
                    st, sp = kc == 0, kc == KC - 1
                    nc.tensor.matmul(pq0, wq_sl(kc, 0, D), xq_sl(xq, kc),
                                     start=st, stop=sp)
                    nc.tensor.matmul(pq1, wq_sl(kc, D, 2 * D),
                                     xq_sl(xq, kc), start=st, stop=sp)
                    nc.tensor.matmul(pk, wkv_sl(kc, 0, D), xq_sl(xq, kc),
                                     start=st, stop=sp)
                    # v directly in [t, d] layout: x^T chunk stationary.
                    # all 4 j-blocks share one PSUM bank: only (kc0, j0)
                    # starts the zero region, only (kc15, j3) stops it
                    for j in range(4):
                        jsl = slice(j * P, (j + 1) * P)
                        nc.tensor.matmul(pv[:, jsl], xq_slj(xq, kc, jsl),
                                         wkv_sl(kc, D, 2 * D),
                                         start=(st and j == 0),
                                         stop=(sp and j == 3))
                prev = (pq0, pq1, pk, pv, slice(0, 512), 0)

                # quarters 1-3: per-tensor chains, interleaved with the
                # previous quarter's rope/v-copy (whose PSUM banks they reuse)
                for qq in range(1, QT):
                    tsl = slice(qq * 512, (qq + 1) * 512)
                    xq = load_xq(qq)
                    emit_prev_rope(prev, 0)
                    pq0, pq1, pk, pv = proj_tiles()
                    for kc in range(KC):
                        nc.tensor.matmul(pk, wkv_sl(kc, 0, D), xq_sl(xq, kc),
                                         start=(kc == 0), stop=(kc == KC - 1))
                    emit_prev_rope(prev, 1)
                    for kc in range(KC):
                        nc.tensor.matmul(pq0, wq_sl(kc, 0, D), xq_sl(xq, kc),
                                         start=(kc == 0), stop=(kc == KC - 1))
                    if qq == QT - 1:
                        # last quarter: rope its own kT mid-quarter so phase
                        # A's first score matmuls aren't gated on the tail
                        rope(pk, kT_t[qq], tsl)
                    else:
                        emit_prev_rope(prev, 2)
                    for kc in range(KC):
                        for j in range(4):
                            jsl = slice(j * P, (j + 1) * P)
                            nc.tensor.matmul(pv[:, jsl], xq_slj(xq, kc, jsl),
                                             wkv_sl(kc, D, 2 * D),
                                             start=(kc == 0 and j == 0),
                                             stop=(kc == KC - 1 and j == 3))
                    if qq == QT - 1:
                        emit_prev_rope(prev, 2)
                        rope(pq0, qT_t[0][qq], tsl)
                    else:
                        emit_prev_rope(prev, 3)
                    for kc in range(KC):
                        nc.tensor.matmul(pq1, wq_sl(kc, D, 2 * D),
                                         xq_sl(xq, kc),
                                         start=(kc == 0), stop=(kc == KC - 1))
                    if qq == QT - 1:
                        emit_prev_rope(prev, 3)
                        nc.scalar.copy(v_t[qq], pv)
                        rope(pq1, qT_t[1][qq], tsl)
                    prev = (pq0, pq1, pk, pv, tsl, qq)

            # ---------------- phase A: attention + chunked o-proj -----------
            with ExitStack() as actx:
                ps_pool = actx.enter_context(
                    tc.tile_pool(name="ps_pool", bufs=2, space="PSUM"))
                pv_pool = actx.enter_context(
                    tc.tile_pool(name="pv_pool", bufs=2, space="PSUM"))
                sums_pool = actx.enter_context(
                    tc.tile_pool(name="sums_pool", bufs=1, space="PSUM"))
                po_pool = actx.enter_context(
                    tc.tile_pool(name="po_pool", bufs=1, space="PSUM"))
                epool = actx.enter_context(tc.tile_pool(name="epool", bufs=6))
                elpool = actx.enter_context(tc.tile_pool(name="elpool", bufs=4))
                spool = actx.enter_context(tc.tile_pool(name="spool", bufs=3))
                bpool = actx.enter_context(tc.tile_pool(name="bpool", bufs=3))
                mpool = actx.enter_context(tc.tile_pool(name="mpool", bufs=3))
                opool = actx.enter_context(tc.tile_pool(name="opool", bufs=6))

                # build the flat chunk-job list: g outer, h inner
                class Grp:
                    pass

                jobs = []
                for g in range(G):
                    for h in range(H_LOC):
                        nkb = 2 * g + 2
                        grp = Grp()
                        grp.g, grp.h, grp.nkb = g, h, nkb
                        grp.kba = max(2 * g - 1, 0)
                        grp.kbb = grp.kba + 1
                        grp.e_map = {}
                        grp.pvacc = None
                        grp.sums = None
                        chunks = []
                        i = 0
                        while nkb - i >= 4:
                            chunks.append(list(range(i, i + 4)))
                            i += 4
                        if i < nkb:
                            chunks.append(list(range(i, i + 2)))
                        for ci, chunk in enumerate(chunks):
                            jobs.append((grp, chunk, ci == 0,
                                         ci == len(chunks) - 1))

                oproj_q = []
                copy_engines = [nc.scalar.copy, nc.vector.tensor_copy]
                copy_i = [0]

                def emit_oproj(tb, cgi, from_ps=False):
                    csl = slice(cgi * 512, (cgi + 1) * 512)
                    if from_ps:
                        # epilogue: score-chunk PSUM banks are dead, rotate po
                        # through them so chains overlap their free-up copies
                        po = ps_pool.tile([P, 1024], F32, tag="ps",
                                          name="ps")[:, 0:512]
                    else:
                        po = po_pool.tile([P, 512], F32, tag="po", name="po")
                    for hh in range(H_LOC):
                        nc.tensor.matmul(po, outT_sl(hh, tb),
                                         wo_sb[:, hh, csl],
                                         start=(hh == 0),
                                         stop=(hh == H_LOC - 1))
                    o_t = opool.tile([P, 512], BF16, tag="o_t", name="o_t")
                    copy_engines[copy_i[0] % 2](o_t, po)
                    copy_i[0] += 1
                    nc.sync.dma_start(
                        out=out[tb * P:(tb + 1) * P, csl], in_=o_t)

                def drain_oproj(n, from_ps=False):
                    for i in range(min(n, len(oproj_q))):
                        emit_oproj(*oproj_q.pop(0), from_ps=from_ps)

                def emit_scores(job):
                    grp, chunk, first, last = job
                    g, h = grp.g, grp.h
                    nkb = grp.nkb
                    if first:
                        grp.pvacc = pv_pool.tile([P, 512], F32, tag="pvacc",
                                                 name="pvacc")
                        grp.sums = sums_pool.tile([1, 512], F32, tag="sums",
                                                  name="sums")
                    ps = ps_pool.tile([P, 1024], F32, tag="ps", name="ps")
                    # the group's final key block (kb = 2g+1) is fully masked
                    # for the first 128 queries: compute only its valid 128
                    # columns. everything else is 256 wide.
                    off = 0
                    for i, kb in enumerate(chunk):
                        wkb = 128 if kb == nkb - 1 else 256
                        qlo = g * 256 + (256 - wkb)
                        nc.tensor.matmul(
                            ps[:, off:off + wkb], kT_sl(kb),
                            qT_t[h][g // 2][:, (g % 2) * 256 + (256 - wkb):
                                            (g % 2 + 1) * 256],
                            start=(i % 2 == 0), stop=(i % 2 == 1))
                        grp.e_map[kb] = (None, off, wkb)
                        off += wkb
                    w = off
                    if last:
                        nc.vector.tensor_add(ps[:, w - 384:w],
                                             ps[:, w - 384:w], maskc)
                    e = epool.tile([P, 1024], BF16, tag="e", name="e")
                    nc.scalar.activation(e[:, 0:w], ps[:, 0:w], Exp)
                    for kb in chunk:
                        _, off_kb, wkb = grp.e_map[kb]
                        grp.e_map[kb] = (e, off_kb, wkb)


                def emit_consume(job):
                    grp, chunk, first, last = job
                    g, h, nkb = grp.g, grp.h, grp.nkb
                    pvacc, sums = grp.pvacc, grp.sums
                    for i, kb in enumerate(chunk):
                        e, off, wkb = grp.e_map[kb]
                        esl = e[:, off:off + wkb]
                        qo = 256 - wkb          # query offset for narrow kb
                        vr = v_sl(kb)
                        nc.tensor.matmul(pvacc[:, qo:256], vr, esl,
                                         start=(kb == 0),
                                         stop=(kb == nkb - 1))
                        nc.tensor.matmul(sums[0:1, qo:256], ones, esl,
                                         start=(kb == 0),
                                         stop=(kb == nkb - 1))
                        if kb in (grp.kba, grp.kbb):
                            # pol/psl share the pog/psg PSUM banks: they ride
                            # on the pending-zero set by pog/psg's start=True
                            # (start=False write-then-accumulate semantics)
                            wi = 0 if kb == grp.kba else 1
                            el = elpool.tile([P, 256], BF16, tag="el",
                                             name="el")
                            nc.vector.tensor_mul(
                                el[:, 0:wkb], esl,
                                mulc[:, wi * 256 + qo:(wi + 1) * 256])
                            nc.tensor.matmul(pvacc[:, 256 + qo:512], vr,
                                             el[:, 0:wkb],
                                             start=False, stop=False,
                                             skip_group_check=True)
                            nc.tensor.matmul(sums[0:1, 256 + qo:512], ones,
                                             el[:, 0:wkb],
                                             start=False, stop=False,
                                             skip_group_check=True)
                    if last:
                        qsl = slice(g * 256, (g + 1) * 256)
                        rec = spool.tile([1, 512], F32, tag="rec", name="rec")
                        nc.vector.reciprocal(rec, sums)
                        bc = bpool.tile([P, 512], F32, tag="bc", name="bc")
                        nc.gpsimd.partition_broadcast(bc, rec)
                        m1 = mpool.tile([P, 512], F32, tag="m1", name="m1")
                        nc.vector.tensor_mul(m1, pvacc, bc)
                        nc.vector.tensor_add(outT_t[h][g],
                                             m1[:, 0:256], m1[:, 256:512])
                        if h == H_LOC - 1:
                            for tb in (2 * g, 2 * g + 1):
                                for cgi in range(4):
                                    oproj_q.append((tb, cgi))

                emit_scores(jobs[0])
                for j in range(1, len(jobs)):
                    emit_scores(jobs[j])
                    # split the o-proj drain around the consume stage so two
                    # po chains never sit back-to-back on the in-order PE
                    # (the second would stall on the first's PSUM-free copy)
                    drain_oproj(1)
                    emit_consume(jobs[j - 1])
                    drain_oproj(1, from_ps=True)
                emit_consume(jobs[-1])
                while oproj_q:
                    drain_oproj(1)
                    drain_oproj(1, from_ps=True)
    return nc


def _build_program():
    if "nc" not in _PROGRAM_CACHE:
        nc = bacc.Bacc("TRN2", target_bir_lowering=False, debug=False,
                       num_devices=N_CORES)
        _emit(nc)
        nc.compile()
        _PROGRAM_CACHE["nc"] = nc
    return _PROGRAM_CACHE["nc"]


def _feed_layout(w):
    """[C, n] weight -> [P, KC * n] with rows = SBUF partitions."""
    n = w.shape[1]
    return np.ascontiguousarray(
        w.reshape(KC, P, n).transpose(1, 0, 2).reshape(P, KC * n))


def _in_maps(x, wq, wk, wv, wo):
    x = np.asarray(x, np.float32).reshape(T, C)
    xT = x.T.astype(BF)
    # xf[q*P + p, kc*512 + t] = xT[kc*128 + p, q*512 + t]
    xf = np.ascontiguousarray(
        xT.reshape(KC, P, QT, 512).transpose(2, 1, 0, 3).reshape(
            QT * P, KC * 512))
    wq = np.asarray(wq, np.float32)
    wk = np.asarray(wk, np.float32)
    wv = np.asarray(wv, np.float32)
    wo = np.asarray(wo, np.float32)
    consts = _host_constants()
    scale = 1.0 / math.sqrt(D)
    wq_s = wq * scale
    maps = []
    for c in range(N_CORES):
        h0 = H_LOC * c
        kv = h0 // (N_HEAD // N_KV_HEAD)
        m = {
            "xf": xf,
            "wqf": _feed_layout(
                wq_s[:, h0 * D:(h0 + H_LOC) * D].astype(BF)),
            "wkvf": _feed_layout(np.concatenate(
                [wk[:, kv * D:(kv + 1) * D], wv[:, kv * D:(kv + 1) * D]],
                axis=1).astype(BF)),
            "wo": np.ascontiguousarray(
                wo[h0 * D:(h0 + H_LOC) * D, :] * 0.5).astype(BF),
        }
        m.update(consts)
        maps.append(m)
    return maps


def _run(inputs, trace=False):
    nc = _build_program()
    maps = _in_maps(inputs["x"], inputs["wq"], inputs["wk"],
                    inputs["wv"], inputs["wo"])
    res = run_bass_kernel_spmd(nc, maps, list(range(N_CORES)), trace=trace)
    total = np.zeros((T, C), np.float64)
    for rm in res.results:
        total += rm["out"].astype(np.float64)
    out = total.astype(np.float32).reshape(1, T, C)
    return out, res


def kernel(x, wq, wk, wv, wo):
    out, _ = _run({"x": x, "wq": wq, "wk": wk, "wv": wv, "wo": wo})
    return out
